# revision 2
# baseline (speedup 1.0000x reference)
"""Trainium2 Bass kernel: GQA causal self-attention with sliding window + sink.

Tensor-parallel over heads across 8 NeuronCores:
  core c owns q-heads {2c, 2c+1} and kv-group c//2.
Per core: fused QKV projection (bf16 matmuls, fp32 accum), RoPE, block-sparse
sliding-window attention (key-major scores, denominator via a ones-column
appended to V^T, normalization folded into a diag(1/denom) matmul that also
transposes o back to [d, t]), AllGather of o over the 8 cores, then a
column-sharded output projection. Host only shards/concatenates.
"""

import numpy as np
import ml_dtypes

import concourse.bass as bass
import concourse.mybir as mybir
import concourse.tile as tile
from concourse import bacc
from concourse import bass2jax

BF = ml_dtypes.bfloat16
F32 = mybir.dt.float32
BF16 = mybir.dt.bfloat16

NCORES = 8
T = 2048
C = 2048
D = 128
H = 16
G = 4
HPC = 2              # heads per core
NT = T // 128        # 16 query/position tiles
NK = C // 128        # 16 contraction chunks
SCALE = 1.0 / float(np.sqrt(D))
ACT_COPY = None      # set after mybir import below


def _build_program():
    """Build + compile the 8-core SPMD bass program. Returns (nc, meta)."""
    nc = bacc.Bacc("TRN2", target_bir_lowering=False, debug=False,
                   num_devices=NCORES)

    Copy = mybir.ActivationFunctionType.Copy
    Exp = mybir.ActivationFunctionType.Exp

    # ---------------- I/O ----------------
    xT_d = nc.dram_tensor("xT", [C, T], BF16, kind="ExternalInput").ap()
    waT_d = nc.dram_tensor("waT", [C, 4 * D], BF16, kind="ExternalInput").ap()
    wpT_d = nc.dram_tensor("wpT", [C, HPC * D], BF16, kind="ExternalInput").ap()
    cc_d = nc.dram_tensor("cc", [128, T], BF16, kind="ExternalInput").ap()
    ss_d = nc.dram_tensor("ss", [128, T], BF16, kind="ExternalInput").ap()
    ksink_d = nc.dram_tensor("ksink", [D, 1], BF16, kind="ExternalInput").ap()
    vsink1_d = nc.dram_tensor("vsink1", [1, D + 1], BF16, kind="ExternalInput").ap()
    bmask_d = nc.dram_tensor("bmask", [128, 256], BF16, kind="ExternalInput").ap()
    smask_d = nc.dram_tensor("smask", [1, 128], BF16, kind="ExternalInput").ap()
    diag01_d = nc.dram_tensor("diag01", [128, 128], BF16, kind="ExternalInput").ap()

    yT_d = nc.dram_tensor("yT", [HPC * D, T], F32, kind="ExternalOutput").ap()

    with tile.TileContext(nc) as tc:
        with tc.tile_pool(name="const", bufs=1) as cpool, \
             tc.tile_pool(name="persist", bufs=1) as pp, \
             tc.tile_pool(name="ps_a", bufs=2, space="PSUM") as ps_a, \
             tc.tile_pool(name="ps_s", bufs=1, space="PSUM") as ps_s, \
             tc.tile_pool(name="ps_att", bufs=3, space="PSUM") as ps_att, \
             tc.tile_pool(name="dram", bufs=1, space="DRAM") as dram:

            # ---- constants ----
            cc_sb = cpool.tile([128, T], BF16)
            ss_sb = cpool.tile([128, T], BF16)
            ksink_sb = cpool.tile([D, 1], BF16)
            vsink1_sb = cpool.tile([1, D + 1], BF16)
            bmask_sb = cpool.tile([128, 256], BF16)
            smask_sb = cpool.tile([1, 128], BF16)
            diag01_sb = cpool.tile([128, 128], BF16)
            nc.sync.dma_start(cc_sb[:], cc_d[:])
            nc.sync.dma_start(ss_sb[:], ss_d[:])
            nc.sync.dma_start(ksink_sb[:], ksink_d[:])
            nc.sync.dma_start(vsink1_sb[:], vsink1_d[:])
            nc.sync.dma_start(bmask_sb[:], bmask_d[:])
            nc.sync.dma_start(smask_sb[:], smask_d[:])
            nc.sync.dma_start(diag01_sb[:], diag01_d[:])

            # persistent activations
            q0r = pp.tile([128, T], BF16)   # roped q head0 [d, t]
            q1r = pp.tile([128, T], BF16)
            kr = pp.tile([128, T], BF16)    # roped k [d, t]
            vt = pp.tile([128, NT * (D + 1)], BF16)   # V'^T: per pos-tile [128, 129]
            og0 = pp.tile([128, T], BF16)   # normalized o head0 [d, t]
            og1 = pp.tile([128, T], BF16)

            # ================= PHASE 1: QKV projection =================
            with tc.tile_pool(name="ph1", bufs=1) as p1, \
                 tc.tile_pool(name="rtmp", bufs=2) as rt:
                xts = p1.tile([128, NK * T], BF16)       # x^T chunks [c128, t]
                wat = p1.tile([128, NK * 4 * D], BF16)   # W_attn^T chunks
                xT_v = xT_d.rearrange("(c p) t -> c p t", p=128)
                waT_v = waT_d.rearrange("(c p) f -> c p f", p=128)
                for kc in range(NK):
                    nc.sync.dma_start(xts[:, kc * T:(kc + 1) * T], xT_v[kc])
                    nc.sync.dma_start(wat[:, kc * 512:(kc + 1) * 512], waT_v[kc])

                # q0, q1, k in [d, t] layout: out[feat, t]
                raws = []
                for f in range(3):
                    raw = p1.tile([128, T], BF16, name=f"raw{f}", tag=f"raw{f}")
                    raws.append(raw)
                    for n in range(4):
                        ph1_ps = ps_a.tile([128, 512], F32, tag="ps_a")
                        for kc in range(NK):
                            nc.tensor.matmul(
                                ph1_ps[:],
                                wat[:, kc * 512 + f * D: kc * 512 + (f + 1) * D],
                                xts[:, kc * T + n * 512: kc * T + (n + 1) * 512],
                                start=(kc == 0), stop=(kc == NK - 1),
                            )
                        nc.scalar.activation(
                            raw[:, n * 512:(n + 1) * 512], ph1_ps[:], Copy)

                # V'^T directly: out[t, d] tiles, lhsT = xT chunk, rhs = Wv cols
                for tt in range(NT):
                    vt_ps = ps_a.tile([128, 128], F32, tag="ps_a")
                    for kc in range(NK):
                        nc.tensor.matmul(
                            vt_ps[:],
                            xts[:, kc * T + tt * 128: kc * T + tt * 128 + 128],
                            wat[:, kc * 512 + 3 * D: kc * 512 + 4 * D],
                            start=(kc == 0), stop=(kc == NK - 1),
                        )
                    nc.scalar.activation(
                        vt[:, tt * (D + 1): tt * (D + 1) + D], vt_ps[:], Copy)
                ones_view = vt.rearrange("p (n e) -> p n e", e=D + 1)[:, :, D:D + 1]
                nc.vector.memset(ones_view, 1.0)

                # ---- RoPE on q0, q1, k ----
                for raw, roped in zip(raws, [q0r, q1r, kr]):
                    swp = rt.tile([128, T], BF16, name="swp", tag="swp")
                    nc.sync.dma_start(swp[0:64, :], raw[64:128, :])
                    nc.sync.dma_start(swp[64:128, :], raw[0:64, :])
                    t1 = rt.tile([128, T], BF16, name="t1", tag="t1")
                    nc.vector.tensor_mul(t1[:], raw[:], cc_sb[:])
                    t2 = rt.tile([128, T], BF16, name="t2", tag="t2")
                    nc.vector.tensor_mul(t2[:], swp[:], ss_sb[:])
                    nc.vector.tensor_add(roped[:], t1[:], t2[:])

            # ================= PHASE 2: attention =================
            with tc.tile_pool(name="att", bufs=2) as ap:
                for h, (qh, og) in enumerate([(q0r, og0), (q1r, og1)]):
                    for qt in range(NT):
                        # position-tiles: [diag] + [edge?] + cleans ascending
                        tiles = [qt]
                        if qt >= 8:
                            tiles.append(qt - 8)
                        tiles += list(range(max(0, qt - 7), qt))
                        n = len(tiles)

                        sps = ps_s.tile([128, 9 * 128], F32, tag="sbig")
                        for s, pt in enumerate(tiles):
                            nc.tensor.matmul(
                                sps[:, s * 128:(s + 1) * 128],
                                kr[:, pt * 128:(pt + 1) * 128],
                                qh[:, qt * 128:(qt + 1) * 128],
                                start=True, stop=True,
                            )
                        expS = ap.tile([128, 9 * 128], BF16, tag="expS")
                        nc.scalar.activation(
                            expS[:, :n * 128], sps[:, :n * 128], Exp, scale=SCALE)
                        # masks: slot 0 = diag triangle; slot 1 = edge (qt>=8)
                        mw = 256 if qt >= 8 else 128
                        nc.vector.tensor_mul(
                            expS[:, :mw], expS[:, :mw], bmask_sb[:, :mw])

                        has_sink = qt <= 7
                        if has_sink:
                            sink_ps = ps_att.tile([1, 128], F32, tag="ps_att")
                            nc.tensor.matmul(
                                sink_ps[0:1, :], ksink_sb[:],
                                qh[:, qt * 128:(qt + 1) * 128],
                                start=True, stop=True,
                            )
                            sink_sb = ap.tile([1, 128], BF16, tag="sink_sb")
                            nc.scalar.activation(
                                sink_sb[0:1, :], sink_ps[0:1, :], Exp, scale=SCALE)
                            if qt == 7:
                                nc.vector.tensor_mul(
                                    sink_sb[0:1, :], sink_sb[0:1, :], smask_sb[0:1, :])

                        o_ps = ps_att.tile([128, D + 1], F32, tag="ps_att")
                        for s, pt in enumerate(tiles):
                            nc.tensor.matmul(
                                o_ps[:],
                                expS[:, s * 128:(s + 1) * 128],
                                vt[:, pt * (D + 1):(pt + 1) * (D + 1)],
                                start=(s == 0),
                                stop=(s == n - 1 and not has_sink),
                            )
                        if has_sink:
                            nc.tensor.matmul(
                                o_ps[:], sink_sb[0:1, :], vsink1_sb[0:1, :],
                                start=False, stop=True,
                            )

                        # normalize + transpose back to [d, t] via diag matmul
                        recip = ap.tile([128, 1], F32, tag="recip")
                        nc.vector.reciprocal(recip[:], o_ps[:, D:D + 1])
                        diagr = ap.tile([128, 128], BF16, tag="diagr")
                        nc.vector.tensor_scalar_mul(diagr[:], diag01_sb[:], recip[:])
                        oT_sb = ap.tile([128, 128], BF16, tag="oT_sb")
                        nc.vector.tensor_copy(oT_sb[:], o_ps[:, 0:D])
                        odt_ps = ps_att.tile([128, 128], F32, tag="ps_att")
                        nc.tensor.matmul(odt_ps[:], oT_sb[:], diagr[:],
                                         start=True, stop=True)
                        nc.scalar.activation(
                            og[:, qt * 128:(qt + 1) * 128], odt_ps[:], Copy)

            # ================= PHASE 3: AllGather o =================
            obounce = dram.tile([HPC * D, T], BF16)
            ogather = dram.tile([NCORES * HPC * D, T], BF16)
            nc.sync.dma_start(obounce[0:128, :], og0[:])
            nc.sync.dma_start(obounce[128:256, :], og1[:])
            nc.gpsimd.collective_compute(
                "AllGather",
                mybir.AluOpType.bypass,
                replica_groups=[list(range(NCORES))],
                ins=[obounce.opt()],
                outs=[ogather.opt()],
            )

            # ================= PHASE 4: output projection =================
            with tc.tile_pool(name="ph4", bufs=1) as p4:
                wpt = p4.tile([128, NK * HPC * D], BF16)
                wpT_v = wpT_d.rearrange("(c p) f -> c p f", p=128)
                og_v = ogather.rearrange("(c p) t -> c p t", p=128)
                ofs = []
                for dc in range(NK):
                    of = p4.tile([128, T], BF16, name=f"of{dc}", tag=f"of{dc}")
                    ofs.append(of)
                    nc.sync.dma_start(of[:], og_v[dc])
                    nc.sync.dma_start(
                        wpt[:, dc * 256:(dc + 1) * 256], wpT_v[dc])
                for m in range(HPC):
                    yt_sb = p4.tile([128, T], F32, name="yt_sb", tag=f"yt{m}")
                    for nn in range(4):
                        y_ps = ps_a.tile([128, 512], F32, tag="ps_a")
                        for dc in range(NK):
                            nc.tensor.matmul(
                                y_ps[:],
                                wpt[:, dc * 256 + m * 128: dc * 256 + (m + 1) * 128],
                                ofs[dc][:, nn * 512:(nn + 1) * 512],
                                start=(dc == 0), stop=(dc == NK - 1),
                            )
                        nc.scalar.activation(
                            yt_sb[:, nn * 512:(nn + 1) * 512], y_ps[:], Copy)
                    nc.sync.dma_start(yT_d[m * 128:(m + 1) * 128, :], yt_sb[:])

    nc.compile()
    return nc


def _prep_inputs(x, cos, sin, W_attn, W_proj, k_sink, v_sink):
    """Host-side sharding: pure slicing / transposition / dtype casts."""
    xT = np.ascontiguousarray(x[0].T).astype(BF)                  # [C, T]
    cosT = cos.T.astype(np.float32)                                # [64, T]
    sinT = sin.T.astype(np.float32)
    cc = np.concatenate([cosT, cosT], axis=0).astype(BF)           # [128, T]
    ss = np.concatenate([-sinT, sinT], axis=0).astype(BF)

    wa = W_attn.reshape(G, 6, D, C)                                # [g, j, d, c]
    ks = k_sink.reshape(G, D)
    vs = v_sink.reshape(G, D)

    kk = np.arange(128)[:, None]
    ii = np.arange(128)[None, :]
    diagm = (kk <= ii).astype(np.float32)
    edgem = (kk >= ii + 1).astype(np.float32)
    bmask = np.concatenate([diagm, edgem], axis=1).astype(BF)      # [128, 256]
    smask = (np.arange(128) <= 126).astype(np.float32).reshape(1, 128).astype(BF)
    diag01 = np.eye(128, dtype=np.float32).astype(BF)

    in_maps = []
    for c in range(NCORES):
        g = c // 2
        j0 = (2 * c) % 4
        waT = np.concatenate(
            [wa[g, j0], wa[g, j0 + 1], wa[g, 4], wa[g, 5]], axis=0,
        ).T.astype(BF)                                             # [C, 512]
        wpT = W_proj[256 * c: 256 * (c + 1), :].T.astype(BF)       # [C, 256]
        ksink = ks[g].reshape(D, 1).astype(BF)
        vsink1 = np.concatenate([vs[g], [1.0]]).reshape(1, D + 1).astype(BF)
        in_maps.append({
            "xT": xT,
            "waT": np.ascontiguousarray(waT),
            "wpT": np.ascontiguousarray(wpT),
            "cc": cc, "ss": ss,
            "ksink": ksink, "vsink1": vsink1,
            "bmask": bmask, "smask": smask, "diag01": diag01,
        })
    return in_maps


# ---------------- cached PJRT runner (modeled on bass2jax.run_bass_via_pjrt) ----
_CACHE = {}


def _get_runner():
    if "runner" in _CACHE:
        return _CACHE["runner"]

    import jax
    from jax.sharding import Mesh, PartitionSpec
    from jax.experimental.shard_map import shard_map

    nc = _build_program()
    bass2jax.install_neuronx_cc_hook()

    partition_name = (nc.partition_id_tensor.name
                      if nc.partition_id_tensor else None)
    in_names = []
    out_names = []
    out_avals = []
    zero_outs = []
    for alloc in nc.m.functions[0].allocations:
        if not isinstance(alloc, mybir.MemoryLocationSet):
            continue
        if not alloc.memorylocations:
            continue
        name = alloc.memorylocations[0].name
        if alloc.kind == "ExternalInput":
            if name != partition_name:
                in_names.append(name)
        elif alloc.kind == "ExternalOutput":
            shape = tuple(alloc.tensor_shape)
            dtype = mybir.dt.np(alloc.dtype)
            out_names.append(name)
            out_avals.append(jax.core.ShapedArray(shape, dtype))
            zero_outs.append(np.zeros(shape, dtype))
    n_params = len(in_names)
    n_outs = len(out_avals)
    all_names = in_names + out_names
    if partition_name is not None:
        all_names = all_names + [partition_name]
    donate = tuple(range(n_params, n_params + n_outs))

    def _body(*args):
        operands = list(args)
        if partition_name is not None:
            operands.append(bass2jax.partition_id_tensor())
        outs = bass2jax._bass_exec_p.bind(
            *operands,
            out_avals=tuple(out_avals),
            in_names=tuple(all_names),
            out_names=tuple(out_names),
            lowering_input_output_aliases=(),
            sim_require_finite=True,
            sim_require_nnan=True,
            nc=nc,
        )
        return tuple(outs)

    devices = jax.devices()[:NCORES]
    mesh = Mesh(np.asarray(devices), ("core",))
    in_specs = (PartitionSpec("core"),) * (n_params + n_outs)
    out_specs = (PartitionSpec("core"),) * n_outs
    sharded = jax.jit(
        shard_map(_body, mesh=mesh, in_specs=in_specs, out_specs=out_specs,
                  check_rep=False),
        donate_argnums=donate, keep_unused=True,
    )

    def run(in_maps):
        concat_in = [
            np.concatenate([np.asarray(in_maps[c][nm]) for c in range(NCORES)],
                           axis=0)
            for nm in in_names
        ]
        concat_zeros = [
            np.zeros((NCORES * z.shape[0], *z.shape[1:]), z.dtype)
            for z in zero_outs
        ]
        out_arrs = sharded(*concat_in, *concat_zeros)
        return [
            {nm: np.asarray(out_arrs[i]).reshape(NCORES, *out_avals[i].shape)[c]
             for i, nm in enumerate(out_names)}
            for c in range(NCORES)
        ]

    _CACHE["runner"] = run
    return run


def kernel(x, cos, sin, W_attn, W_proj, k_sink, v_sink):
    run = _get_runner()
    in_maps = _prep_inputs(x, cos, sin, W_attn, W_proj, k_sink, v_sink)
    results = run(in_maps)
    y = np.empty((T, C), dtype=np.float32)
    for c in range(NCORES):
        y[:, 256 * c: 256 * (c + 1)] = results[c]["yT"].T
    return y.reshape(1, T, C)


# revision 3
# speedup vs baseline: 29.3196x; 29.3196x over previous
"""Trainium2 Bass kernel: GQA causal self-attention with sliding window + sink.

Tensor-parallel over heads across 8 NeuronCores:
  core c owns q-heads {2c, 2c+1} and kv-group c//2.
Per core: fused QKV projection (bf16 matmuls, fp32 accum), RoPE, block-sparse
sliding-window attention (key-major scores, denominator via a ones-column
appended to V^T, normalization folded into a diag(1/denom) matmul that also
transposes o back to [d, t]), AllGather of o over the 8 cores, then a
column-sharded output projection. Host only shards/concatenates.
"""

import numpy as np
import ml_dtypes

import concourse.bass as bass
import concourse.mybir as mybir
import concourse.tile as tile
from concourse import bacc
from concourse import bass2jax

BF = ml_dtypes.bfloat16
F32 = mybir.dt.float32
BF16 = mybir.dt.bfloat16

NCORES = 8
T = 2048
C = 2048
D = 128
H = 16
G = 4
HPC = 2              # heads per core
NT = T // 128        # 16 query/position tiles
NK = C // 128        # 16 contraction chunks
SCALE = 1.0 / float(np.sqrt(D))
ACT_COPY = None      # set after mybir import below


def _build_program():
    """Build + compile the 8-core SPMD bass program. Returns (nc, meta)."""
    nc = bacc.Bacc("TRN2", target_bir_lowering=False, debug=False,
                   num_devices=NCORES)

    Copy = mybir.ActivationFunctionType.Copy
    Exp = mybir.ActivationFunctionType.Exp

    # ---------------- I/O ----------------
    xT_d = nc.dram_tensor("xT", [C, T], BF16, kind="ExternalInput").ap()
    waT_d = nc.dram_tensor("waT", [C, 4 * D], BF16, kind="ExternalInput").ap()
    wpT_d = nc.dram_tensor("wpT", [C, HPC * D], BF16, kind="ExternalInput").ap()
    cc_d = nc.dram_tensor("cc", [128, T], BF16, kind="ExternalInput").ap()
    ss_d = nc.dram_tensor("ss", [128, T], BF16, kind="ExternalInput").ap()
    ksink_d = nc.dram_tensor("ksink", [D, 1], BF16, kind="ExternalInput").ap()
    vsink1_d = nc.dram_tensor("vsink1", [1, D + 1], BF16, kind="ExternalInput").ap()
    bmask_d = nc.dram_tensor("bmask", [128, 256], BF16, kind="ExternalInput").ap()
    smask_d = nc.dram_tensor("smask", [1, 128], BF16, kind="ExternalInput").ap()
    diag01_d = nc.dram_tensor("diag01", [128, 128], BF16, kind="ExternalInput").ap()

    yT_d = nc.dram_tensor("yT", [HPC * D, T], F32, kind="ExternalOutput").ap()

    with tile.TileContext(nc) as tc:
        with tc.tile_pool(name="const", bufs=1) as cpool, \
             tc.tile_pool(name="persist", bufs=1) as pp, \
             tc.tile_pool(name="ps_a", bufs=2, space="PSUM") as ps_a, \
             tc.tile_pool(name="ps_s", bufs=1, space="PSUM") as ps_s, \
             tc.tile_pool(name="ps_att", bufs=3, space="PSUM") as ps_att, \
             tc.tile_pool(name="dram", bufs=1, space="DRAM") as dram:

            # ---- constants ----
            cc_sb = cpool.tile([128, T], BF16)
            ss_sb = cpool.tile([128, T], BF16)
            ksink_sb = cpool.tile([D, 1], BF16)
            vsink1_sb = cpool.tile([1, D + 1], BF16)
            bmask_sb = cpool.tile([128, 256], BF16)
            smask_sb = cpool.tile([1, 128], BF16)
            diag01_sb = cpool.tile([128, 128], BF16)
            nc.sync.dma_start(cc_sb[:], cc_d[:])
            nc.sync.dma_start(ss_sb[:], ss_d[:])
            nc.sync.dma_start(ksink_sb[:], ksink_d[:])
            nc.sync.dma_start(vsink1_sb[:], vsink1_d[:])
            nc.sync.dma_start(bmask_sb[:], bmask_d[:])
            nc.sync.dma_start(smask_sb[:], smask_d[:])
            nc.sync.dma_start(diag01_sb[:], diag01_d[:])

            # persistent activations
            q0r = pp.tile([128, T], BF16)   # roped q head0 [d, t]
            q1r = pp.tile([128, T], BF16)
            kr = pp.tile([128, T], BF16)    # roped k [d, t]
            vt = pp.tile([128, NT * (D + 1)], BF16)   # V'^T: per pos-tile [128, 129]
            og0 = pp.tile([128, T], BF16)   # normalized o head0 [d, t]
            og1 = pp.tile([128, T], BF16)

            # ================= PHASE 1: QKV projection =================
            with tc.tile_pool(name="ph1", bufs=1) as p1, \
                 tc.tile_pool(name="rtmp", bufs=2) as rt:
                xts = p1.tile([128, NK * T], BF16)       # x^T chunks [c128, t]
                wat = p1.tile([128, NK * 4 * D], BF16)   # W_attn^T chunks
                xT_v = xT_d.rearrange("(c p) t -> c p t", p=128)
                waT_v = waT_d.rearrange("(c p) f -> c p f", p=128)
                for kc in range(NK):
                    nc.sync.dma_start(xts[:, kc * T:(kc + 1) * T], xT_v[kc])
                    nc.sync.dma_start(wat[:, kc * 512:(kc + 1) * 512], waT_v[kc])

                # q0, q1, k in [d, t] layout: out[feat, t]
                raws = []
                for f in range(3):
                    raw = p1.tile([128, T], BF16, name=f"raw{f}", tag=f"raw{f}")
                    raws.append(raw)
                    for n in range(4):
                        ph1_ps = ps_a.tile([128, 512], F32, tag="ps_a")
                        for kc in range(NK):
                            nc.tensor.matmul(
                                ph1_ps[:],
                                wat[:, kc * 512 + f * D: kc * 512 + (f + 1) * D],
                                xts[:, kc * T + n * 512: kc * T + (n + 1) * 512],
                                start=(kc == 0), stop=(kc == NK - 1),
                            )
                        nc.scalar.activation(
                            raw[:, n * 512:(n + 1) * 512], ph1_ps[:], Copy)

                # V'^T directly: out[t, d] tiles, lhsT = xT chunk, rhs = Wv cols
                for tt in range(NT):
                    vt_ps = ps_a.tile([128, 128], F32, tag="ps_a")
                    for kc in range(NK):
                        nc.tensor.matmul(
                            vt_ps[:],
                            xts[:, kc * T + tt * 128: kc * T + tt * 128 + 128],
                            wat[:, kc * 512 + 3 * D: kc * 512 + 4 * D],
                            start=(kc == 0), stop=(kc == NK - 1),
                        )
                    nc.scalar.activation(
                        vt[:, tt * (D + 1): tt * (D + 1) + D], vt_ps[:], Copy)
                ones_view = vt.rearrange("p (n e) -> p n e", e=D + 1)[:, :, D:D + 1]
                nc.vector.memset(ones_view, 1.0)

                # ---- RoPE on q0, q1, k ----
                for raw, roped in zip(raws, [q0r, q1r, kr]):
                    swp = rt.tile([128, T], BF16, name="swp", tag="swp")
                    nc.sync.dma_start(swp[0:64, :], raw[64:128, :])
                    nc.sync.dma_start(swp[64:128, :], raw[0:64, :])
                    t1 = rt.tile([128, T], BF16, name="t1", tag="t1")
                    nc.vector.tensor_mul(t1[:], raw[:], cc_sb[:])
                    t2 = rt.tile([128, T], BF16, name="t2", tag="t2")
                    nc.vector.tensor_mul(t2[:], swp[:], ss_sb[:])
                    nc.vector.tensor_add(roped[:], t1[:], t2[:])

            # ================= PHASE 2: attention =================
            with tc.tile_pool(name="att", bufs=2) as ap:
                for h, (qh, og) in enumerate([(q0r, og0), (q1r, og1)]):
                    for qt in range(NT):
                        # position-tiles: [diag] + [edge?] + cleans ascending
                        tiles = [qt]
                        if qt >= 8:
                            tiles.append(qt - 8)
                        tiles += list(range(max(0, qt - 7), qt))
                        n = len(tiles)

                        sps = ps_s.tile([128, 9 * 128], F32, tag="sbig")
                        for s, pt in enumerate(tiles):
                            nc.tensor.matmul(
                                sps[:, s * 128:(s + 1) * 128],
                                kr[:, pt * 128:(pt + 1) * 128],
                                qh[:, qt * 128:(qt + 1) * 128],
                                start=True, stop=True,
                            )
                        expS = ap.tile([128, 9 * 128], BF16, tag="expS")
                        nc.scalar.activation(
                            expS[:, :n * 128], sps[:, :n * 128], Exp, scale=SCALE)
                        # masks: slot 0 = diag triangle; slot 1 = edge (qt>=8)
                        mw = 256 if qt >= 8 else 128
                        nc.vector.tensor_mul(
                            expS[:, :mw], expS[:, :mw], bmask_sb[:, :mw])

                        has_sink = qt <= 7
                        if has_sink:
                            sink_ps = ps_att.tile([1, 128], F32, tag="ps_att")
                            nc.tensor.matmul(
                                sink_ps[0:1, :], ksink_sb[:],
                                qh[:, qt * 128:(qt + 1) * 128],
                                start=True, stop=True,
                            )
                            sink_sb = ap.tile([1, 128], BF16, tag="sink_sb")
                            nc.scalar.activation(
                                sink_sb[0:1, :], sink_ps[0:1, :], Exp, scale=SCALE)
                            if qt == 7:
                                nc.vector.tensor_mul(
                                    sink_sb[0:1, :], sink_sb[0:1, :], smask_sb[0:1, :])

                        o_ps = ps_att.tile([128, D + 1], F32, tag="ps_att")
                        for s, pt in enumerate(tiles):
                            nc.tensor.matmul(
                                o_ps[:],
                                expS[:, s * 128:(s + 1) * 128],
                                vt[:, pt * (D + 1):(pt + 1) * (D + 1)],
                                start=(s == 0),
                                stop=(s == n - 1 and not has_sink),
                            )
                        if has_sink:
                            nc.tensor.matmul(
                                o_ps[:], sink_sb[0:1, :], vsink1_sb[0:1, :],
                                start=False, stop=True,
                            )

                        # normalize + transpose back to [d, t] via diag matmul
                        recip = ap.tile([128, 1], F32, tag="recip")
                        nc.vector.reciprocal(recip[:], o_ps[:, D:D + 1])
                        diagr = ap.tile([128, 128], BF16, tag="diagr")
                        nc.vector.tensor_scalar_mul(diagr[:], diag01_sb[:], recip[:])
                        oT_sb = ap.tile([128, 128], BF16, tag="oT_sb")
                        nc.vector.tensor_copy(oT_sb[:], o_ps[:, 0:D])
                        odt_ps = ps_att.tile([128, 128], F32, tag="ps_att")
                        nc.tensor.matmul(odt_ps[:], oT_sb[:], diagr[:],
                                         start=True, stop=True)
                        nc.scalar.activation(
                            og[:, qt * 128:(qt + 1) * 128], odt_ps[:], Copy)

            # ================= PHASE 3: AllGather o =================
            obounce = dram.tile([HPC * D, T], BF16)
            ogather = dram.tile([NCORES * HPC * D, T], BF16)
            nc.sync.dma_start(obounce[0:128, :], og0[:])
            nc.sync.dma_start(obounce[128:256, :], og1[:])
            nc.gpsimd.collective_compute(
                "AllGather",
                mybir.AluOpType.bypass,
                replica_groups=[list(range(NCORES))],
                ins=[obounce.opt()],
                outs=[ogather.opt()],
            )

            # ================= PHASE 4: output projection =================
            with tc.tile_pool(name="ph4", bufs=1) as p4:
                wpt = p4.tile([128, NK * HPC * D], BF16)
                wpT_v = wpT_d.rearrange("(c p) f -> c p f", p=128)
                og_v = ogather.rearrange("(c p) t -> c p t", p=128)
                ofs = []
                for dc in range(NK):
                    of = p4.tile([128, T], BF16, name=f"of{dc}", tag=f"of{dc}")
                    ofs.append(of)
                    nc.sync.dma_start(of[:], og_v[dc])
                    nc.sync.dma_start(
                        wpt[:, dc * 256:(dc + 1) * 256], wpT_v[dc])
                for m in range(HPC):
                    yt_sb = p4.tile([128, T], F32, name="yt_sb", tag=f"yt{m}")
                    for nn in range(4):
                        y_ps = ps_a.tile([128, 512], F32, tag="ps_a")
                        for dc in range(NK):
                            nc.tensor.matmul(
                                y_ps[:],
                                wpt[:, dc * 256 + m * 128: dc * 256 + (m + 1) * 128],
                                ofs[dc][:, nn * 512:(nn + 1) * 512],
                                start=(dc == 0), stop=(dc == NK - 1),
                            )
                        nc.scalar.activation(
                            yt_sb[:, nn * 512:(nn + 1) * 512], y_ps[:], Copy)
                    nc.sync.dma_start(yT_d[m * 128:(m + 1) * 128, :], yt_sb[:])

    nc.compile()
    return nc


def _prep_inputs(x, cos, sin, W_attn, W_proj, k_sink, v_sink):
    """Host-side sharding: pure slicing / transposition / dtype casts."""
    xT = np.ascontiguousarray(x[0].T).astype(BF)                  # [C, T]
    cosT = cos.T.astype(np.float32)                                # [64, T]
    sinT = sin.T.astype(np.float32)
    cc = np.concatenate([cosT, cosT], axis=0).astype(BF)           # [128, T]
    ss = np.concatenate([-sinT, sinT], axis=0).astype(BF)

    wa = W_attn.reshape(G, 6, D, C)                                # [g, j, d, c]
    ks = k_sink.reshape(G, D)
    vs = v_sink.reshape(G, D)

    kk = np.arange(128)[:, None]
    ii = np.arange(128)[None, :]
    diagm = (kk <= ii).astype(np.float32)
    edgem = (kk >= ii + 1).astype(np.float32)
    bmask = np.concatenate([diagm, edgem], axis=1).astype(BF)      # [128, 256]
    smask = (np.arange(128) <= 126).astype(np.float32).reshape(1, 128).astype(BF)
    diag01 = np.eye(128, dtype=np.float32).astype(BF)

    in_maps = []
    for c in range(NCORES):
        g = c // 2
        j0 = (2 * c) % 4
        waT = np.concatenate(
            [wa[g, j0], wa[g, j0 + 1], wa[g, 4], wa[g, 5]], axis=0,
        ).T.astype(BF)                                             # [C, 512]
        wpT = W_proj[256 * c: 256 * (c + 1), :].T.astype(BF)       # [C, 256]
        ksink = ks[g].reshape(D, 1).astype(BF)
        vsink1 = np.concatenate([vs[g], [1.0]]).reshape(1, D + 1).astype(BF)
        in_maps.append({
            "xT": xT,
            "waT": np.ascontiguousarray(waT),
            "wpT": np.ascontiguousarray(wpT),
            "cc": cc, "ss": ss,
            "ksink": ksink, "vsink1": vsink1,
            "bmask": bmask, "smask": smask, "diag01": diag01,
        })
    return in_maps


# ---------------- cached PJRT runner (modeled on bass2jax.run_bass_via_pjrt) ----
_CACHE = {}


def _get_runner():
    if "runner" in _CACHE:
        return _CACHE["runner"]

    import jax
    from jax.sharding import Mesh, PartitionSpec
    from jax.experimental.shard_map import shard_map

    nc = _build_program()
    bass2jax.install_neuronx_cc_hook()

    partition_name = (nc.partition_id_tensor.name
                      if nc.partition_id_tensor else None)
    in_names = []
    out_names = []
    out_avals = []
    zero_outs = []
    for alloc in nc.m.functions[0].allocations:
        if not isinstance(alloc, mybir.MemoryLocationSet):
            continue
        if not alloc.memorylocations:
            continue
        name = alloc.memorylocations[0].name
        if alloc.kind == "ExternalInput":
            if name != partition_name:
                in_names.append(name)
        elif alloc.kind == "ExternalOutput":
            shape = tuple(alloc.tensor_shape)
            dtype = mybir.dt.np(alloc.dtype)
            out_names.append(name)
            out_avals.append(jax.core.ShapedArray(shape, dtype))
            zero_outs.append(np.zeros(shape, dtype))
    n_params = len(in_names)
    n_outs = len(out_avals)
    all_names = in_names + out_names
    if partition_name is not None:
        all_names = all_names + [partition_name]
    donate = tuple(range(n_params, n_params + n_outs))

    def _body(*args):
        operands = list(args)
        if partition_name is not None:
            operands.append(bass2jax.partition_id_tensor())
        outs = bass2jax._bass_exec_p.bind(
            *operands,
            out_avals=tuple(out_avals),
            in_names=tuple(all_names),
            out_names=tuple(out_names),
            lowering_input_output_aliases=(),
            sim_require_finite=True,
            sim_require_nnan=True,
            nc=nc,
        )
        return tuple(outs)

    devices = jax.devices()[:NCORES]
    mesh = Mesh(np.asarray(devices), ("core",))
    in_specs = (PartitionSpec("core"),) * (n_params + n_outs)
    out_specs = (PartitionSpec("core"),) * n_outs
    sharded = jax.jit(
        shard_map(_body, mesh=mesh, in_specs=in_specs, out_specs=out_specs,
                  check_rep=False),
        donate_argnums=donate, keep_unused=True,
    )

    def run(in_maps):
        concat_in = [
            np.concatenate([np.asarray(in_maps[c][nm]) for c in range(NCORES)],
                           axis=0)
            for nm in in_names
        ]
        concat_zeros = [
            np.zeros((NCORES * z.shape[0], *z.shape[1:]), z.dtype)
            for z in zero_outs
        ]
        out_arrs = sharded(*concat_in, *concat_zeros)
        return [
            {nm: np.asarray(out_arrs[i]).reshape(NCORES, *out_avals[i].shape)[c]
             for i, nm in enumerate(out_names)}
            for c in range(NCORES)
        ]

    _CACHE["runner"] = run
    _CACHE["internals"] = {
        "nc": nc, "sharded": sharded, "mesh": mesh,
        "in_names": in_names, "out_names": out_names,
        "out_avals": out_avals, "zero_outs": zero_outs,
    }
    return run


def kernel(x, cos, sin, W_attn, W_proj, k_sink, v_sink):
    run = _get_runner()
    in_maps = _prep_inputs(x, cos, sin, W_attn, W_proj, k_sink, v_sink)
    results = run(in_maps)
    y = np.empty((T, C), dtype=np.float32)
    for c in range(NCORES):
        y[:, 256 * c: 256 * (c + 1)] = results[c]["yT"].T
    return y.reshape(1, T, C)


# revision 4
# speedup vs baseline: 4504.1173x; 153.6211x over previous
"""Trainium2 Bass kernel: GQA causal self-attention with sliding window + sink.

Tensor-parallel over heads across 8 NeuronCores:
  core c owns q-heads {2c, 2c+1} and kv-group c//2.
Per core: fused QKV projection (bf16 matmuls, fp32 accum), RoPE, block-sparse
sliding-window attention (key-major scores, denominator via a ones-column
appended to V^T, normalization folded into a diag(1/denom) matmul that also
transposes o back to [d, t]), AllGather of o over the 8 cores, then a
column-sharded output projection. Host only shards/concatenates.
"""

import numpy as np
import ml_dtypes

import concourse.bass as bass
import concourse.mybir as mybir
import concourse.tile as tile
from concourse import bacc
from concourse import bass2jax

BF = ml_dtypes.bfloat16
F32 = mybir.dt.float32
BF16 = mybir.dt.bfloat16

NCORES = 8
T = 2048
C = 2048
D = 128
H = 16
G = 4
HPC = 2              # heads per core
NT = T // 128        # 16 query/position tiles
NK = C // 128        # 16 contraction chunks
SCALE = 1.0 / float(np.sqrt(D))
ACT_COPY = None      # set after mybir import below


def _build_program():
    """Build + compile the 8-core SPMD bass program. Returns (nc, meta)."""
    nc = bacc.Bacc("TRN2", target_bir_lowering=False, debug=False,
                   num_devices=NCORES)

    Copy = mybir.ActivationFunctionType.Copy
    Exp = mybir.ActivationFunctionType.Exp

    # ---------------- I/O ----------------
    xT_d = nc.dram_tensor("xT", [C, T], BF16, kind="ExternalInput").ap()
    waT_d = nc.dram_tensor("waT", [C, 4 * D], BF16, kind="ExternalInput").ap()
    wpT_d = nc.dram_tensor("wpT", [C, HPC * D], BF16, kind="ExternalInput").ap()
    cc_d = nc.dram_tensor("cc", [128, T], BF16, kind="ExternalInput").ap()
    ss_d = nc.dram_tensor("ss", [128, T], BF16, kind="ExternalInput").ap()
    ksink_d = nc.dram_tensor("ksink", [D, 1], BF16, kind="ExternalInput").ap()
    vsink1_d = nc.dram_tensor("vsink1", [1, D + 1], BF16, kind="ExternalInput").ap()
    bmask_d = nc.dram_tensor("bmask", [128, 256], BF16, kind="ExternalInput").ap()
    smask_d = nc.dram_tensor("smask", [1, 128], BF16, kind="ExternalInput").ap()
    diag01_d = nc.dram_tensor("diag01", [128, 128], BF16, kind="ExternalInput").ap()

    yT_d = nc.dram_tensor("yT", [HPC * D, T], F32, kind="ExternalOutput").ap()

    with tile.TileContext(nc) as tc:
        with tc.tile_pool(name="const", bufs=1) as cpool, \
             tc.tile_pool(name="persist", bufs=1) as pp, \
             tc.tile_pool(name="ps_a", bufs=2, space="PSUM") as ps_a, \
             tc.tile_pool(name="ps_s", bufs=1, space="PSUM") as ps_s, \
             tc.tile_pool(name="ps_att", bufs=3, space="PSUM") as ps_att, \
             tc.tile_pool(name="dram", bufs=1, space="DRAM") as dram:

            # ---- constants ----
            cc_sb = cpool.tile([128, T], BF16)
            ss_sb = cpool.tile([128, T], BF16)
            ksink_sb = cpool.tile([D, 1], BF16)
            vsink1_sb = cpool.tile([1, D + 1], BF16)
            bmask_sb = cpool.tile([128, 256], BF16)
            smask_sb = cpool.tile([1, 128], BF16)
            diag01_sb = cpool.tile([128, 128], BF16)
            nc.sync.dma_start(cc_sb[:], cc_d[:])
            nc.sync.dma_start(ss_sb[:], ss_d[:])
            nc.sync.dma_start(ksink_sb[:], ksink_d[:])
            nc.sync.dma_start(vsink1_sb[:], vsink1_d[:])
            nc.sync.dma_start(bmask_sb[:], bmask_d[:])
            nc.sync.dma_start(smask_sb[:], smask_d[:])
            nc.sync.dma_start(diag01_sb[:], diag01_d[:])

            # persistent activations
            q0r = pp.tile([128, T], BF16)   # roped q head0 [d, t]
            q1r = pp.tile([128, T], BF16)
            kr = pp.tile([128, T], BF16)    # roped k [d, t]
            vt = pp.tile([128, NT * (D + 1)], BF16)   # V'^T: per pos-tile [128, 129]
            og0 = pp.tile([128, T], BF16)   # normalized o head0 [d, t]
            og1 = pp.tile([128, T], BF16)

            # ================= PHASE 1: QKV projection =================
            with tc.tile_pool(name="ph1", bufs=1) as p1, \
                 tc.tile_pool(name="rtmp", bufs=2) as rt:
                xts = p1.tile([128, NK * T], BF16)       # x^T chunks [c128, t]
                wat = p1.tile([128, NK * 4 * D], BF16)   # W_attn^T chunks
                xT_v = xT_d.rearrange("(c p) t -> c p t", p=128)
                waT_v = waT_d.rearrange("(c p) f -> c p f", p=128)
                for kc in range(NK):
                    nc.sync.dma_start(xts[:, kc * T:(kc + 1) * T], xT_v[kc])
                    nc.sync.dma_start(wat[:, kc * 512:(kc + 1) * 512], waT_v[kc])

                # q0, q1, k in [d, t] layout: out[feat, t]
                raws = []
                for f in range(3):
                    raw = p1.tile([128, T], BF16, name=f"raw{f}", tag=f"raw{f}")
                    raws.append(raw)
                    for n in range(4):
                        ph1_ps = ps_a.tile([128, 512], F32, tag="ps_a")
                        for kc in range(NK):
                            nc.tensor.matmul(
                                ph1_ps[:],
                                wat[:, kc * 512 + f * D: kc * 512 + (f + 1) * D],
                                xts[:, kc * T + n * 512: kc * T + (n + 1) * 512],
                                start=(kc == 0), stop=(kc == NK - 1),
                            )
                        nc.scalar.activation(
                            raw[:, n * 512:(n + 1) * 512], ph1_ps[:], Copy)

                # V'^T directly: out[t, d] tiles, lhsT = xT chunk, rhs = Wv cols
                for tt in range(NT):
                    vt_ps = ps_a.tile([128, 128], F32, tag="ps_a")
                    for kc in range(NK):
                        nc.tensor.matmul(
                            vt_ps[:],
                            xts[:, kc * T + tt * 128: kc * T + tt * 128 + 128],
                            wat[:, kc * 512 + 3 * D: kc * 512 + 4 * D],
                            start=(kc == 0), stop=(kc == NK - 1),
                        )
                    nc.scalar.activation(
                        vt[:, tt * (D + 1): tt * (D + 1) + D], vt_ps[:], Copy)
                ones_view = vt.rearrange("p (n e) -> p n e", e=D + 1)[:, :, D:D + 1]
                nc.vector.memset(ones_view, 1.0)

                # ---- RoPE on q0, q1, k ----
                for raw, roped in zip(raws, [q0r, q1r, kr]):
                    swp = rt.tile([128, T], BF16, name="swp", tag="swp")
                    nc.sync.dma_start(swp[0:64, :], raw[64:128, :])
                    nc.sync.dma_start(swp[64:128, :], raw[0:64, :])
                    t1 = rt.tile([128, T], BF16, name="t1", tag="t1")
                    nc.vector.tensor_mul(t1[:], raw[:], cc_sb[:])
                    t2 = rt.tile([128, T], BF16, name="t2", tag="t2")
                    nc.vector.tensor_mul(t2[:], swp[:], ss_sb[:])
                    nc.vector.tensor_add(roped[:], t1[:], t2[:])

            # ================= PHASE 2: attention =================
            with tc.tile_pool(name="att", bufs=2) as ap:
                for h, (qh, og) in enumerate([(q0r, og0), (q1r, og1)]):
                    for qt in range(NT):
                        # position-tiles: [diag] + [edge?] + cleans ascending
                        tiles = [qt]
                        if qt >= 8:
                            tiles.append(qt - 8)
                        tiles += list(range(max(0, qt - 7), qt))
                        n = len(tiles)

                        sps = ps_s.tile([128, 9 * 128], F32, tag="sbig")
                        for s, pt in enumerate(tiles):
                            nc.tensor.matmul(
                                sps[:, s * 128:(s + 1) * 128],
                                kr[:, pt * 128:(pt + 1) * 128],
                                qh[:, qt * 128:(qt + 1) * 128],
                                start=True, stop=True,
                            )
                        expS = ap.tile([128, 9 * 128], BF16, tag="expS")
                        nc.scalar.activation(
                            expS[:, :n * 128], sps[:, :n * 128], Exp, scale=SCALE)
                        # masks: slot 0 = diag triangle; slot 1 = edge (qt>=8)
                        mw = 256 if qt >= 8 else 128
                        nc.vector.tensor_mul(
                            expS[:, :mw], expS[:, :mw], bmask_sb[:, :mw])

                        has_sink = qt <= 7
                        if has_sink:
                            sink_ps = ps_att.tile([1, 128], F32, tag="ps_att")
                            nc.tensor.matmul(
                                sink_ps[0:1, :], ksink_sb[:],
                                qh[:, qt * 128:(qt + 1) * 128],
                                start=True, stop=True,
                            )
                            sink_sb = ap.tile([1, 128], BF16, tag="sink_sb")
                            nc.scalar.activation(
                                sink_sb[0:1, :], sink_ps[0:1, :], Exp, scale=SCALE)
                            if qt == 7:
                                nc.vector.tensor_mul(
                                    sink_sb[0:1, :], sink_sb[0:1, :], smask_sb[0:1, :])

                        o_ps = ps_att.tile([128, D + 1], F32, tag="ps_att")
                        for s, pt in enumerate(tiles):
                            nc.tensor.matmul(
                                o_ps[:],
                                expS[:, s * 128:(s + 1) * 128],
                                vt[:, pt * (D + 1):(pt + 1) * (D + 1)],
                                start=(s == 0),
                                stop=(s == n - 1 and not has_sink),
                            )
                        if has_sink:
                            nc.tensor.matmul(
                                o_ps[:], sink_sb[0:1, :], vsink1_sb[0:1, :],
                                start=False, stop=True,
                            )

                        # normalize + transpose back to [d, t] via diag matmul
                        recip = ap.tile([128, 1], F32, tag="recip")
                        nc.vector.reciprocal(recip[:], o_ps[:, D:D + 1])
                        diagr = ap.tile([128, 128], BF16, tag="diagr")
                        nc.vector.tensor_scalar_mul(diagr[:], diag01_sb[:], recip[:])
                        oT_sb = ap.tile([128, 128], BF16, tag="oT_sb")
                        nc.vector.tensor_copy(oT_sb[:], o_ps[:, 0:D])
                        odt_ps = ps_att.tile([128, 128], F32, tag="ps_att")
                        nc.tensor.matmul(odt_ps[:], oT_sb[:], diagr[:],
                                         start=True, stop=True)
                        nc.scalar.activation(
                            og[:, qt * 128:(qt + 1) * 128], odt_ps[:], Copy)

            # ================= PHASE 3: AllGather o =================
            obounce = dram.tile([HPC * D, T], BF16)
            ogather = dram.tile([NCORES * HPC * D, T], BF16)
            nc.sync.dma_start(obounce[0:128, :], og0[:])
            nc.sync.dma_start(obounce[128:256, :], og1[:])
            nc.gpsimd.collective_compute(
                "AllGather",
                mybir.AluOpType.bypass,
                replica_groups=[list(range(NCORES))],
                ins=[obounce.opt()],
                outs=[ogather.opt()],
            )

            # ================= PHASE 4: output projection =================
            with tc.tile_pool(name="ph4", bufs=1) as p4:
                wpt = p4.tile([128, NK * HPC * D], BF16)
                wpT_v = wpT_d.rearrange("(c p) f -> c p f", p=128)
                og_v = ogather.rearrange("(c p) t -> c p t", p=128)
                ofs = []
                for dc in range(NK):
                    of = p4.tile([128, T], BF16, name=f"of{dc}", tag=f"of{dc}")
                    ofs.append(of)
                    nc.sync.dma_start(of[:], og_v[dc])
                    nc.sync.dma_start(
                        wpt[:, dc * 256:(dc + 1) * 256], wpT_v[dc])
                for m in range(HPC):
                    yt_sb = p4.tile([128, T], F32, name="yt_sb", tag=f"yt{m}")
                    for nn in range(4):
                        y_ps = ps_a.tile([128, 512], F32, tag="ps_a")
                        for dc in range(NK):
                            nc.tensor.matmul(
                                y_ps[:],
                                wpt[:, dc * 256 + m * 128: dc * 256 + (m + 1) * 128],
                                ofs[dc][:, nn * 512:(nn + 1) * 512],
                                start=(dc == 0), stop=(dc == NK - 1),
                            )
                        nc.scalar.activation(
                            yt_sb[:, nn * 512:(nn + 1) * 512], y_ps[:], Copy)
                    nc.sync.dma_start(yT_d[m * 128:(m + 1) * 128, :], yt_sb[:])

    nc.compile()
    return nc


def _prep_inputs(x, cos, sin, W_attn, W_proj, k_sink, v_sink):
    """Host-side sharding: pure slicing / transposition / dtype casts."""
    xT = np.ascontiguousarray(x[0].T).astype(BF)                  # [C, T]
    cosT = cos.T.astype(np.float32)                                # [64, T]
    sinT = sin.T.astype(np.float32)
    cc = np.concatenate([cosT, cosT], axis=0).astype(BF)           # [128, T]
    ss = np.concatenate([-sinT, sinT], axis=0).astype(BF)

    wa = W_attn.reshape(G, 6, D, C)                                # [g, j, d, c]
    ks = k_sink.reshape(G, D)
    vs = v_sink.reshape(G, D)

    kk = np.arange(128)[:, None]
    ii = np.arange(128)[None, :]
    diagm = (kk <= ii).astype(np.float32)
    edgem = (kk >= ii + 1).astype(np.float32)
    bmask = np.concatenate([diagm, edgem], axis=1).astype(BF)      # [128, 256]
    smask = (np.arange(128) <= 126).astype(np.float32).reshape(1, 128).astype(BF)
    diag01 = np.eye(128, dtype=np.float32).astype(BF)

    in_maps = []
    for c in range(NCORES):
        g = c // 2
        j0 = (2 * c) % 4
        waT = np.concatenate(
            [wa[g, j0], wa[g, j0 + 1], wa[g, 4], wa[g, 5]], axis=0,
        ).T.astype(BF)                                             # [C, 512]
        wpT = W_proj[256 * c: 256 * (c + 1), :].T.astype(BF)       # [C, 256]
        ksink = ks[g].reshape(D, 1).astype(BF)
        vsink1 = np.concatenate([vs[g], [1.0]]).reshape(1, D + 1).astype(BF)
        in_maps.append({
            "xT": xT,
            "waT": np.ascontiguousarray(waT),
            "wpT": np.ascontiguousarray(wpT),
            "cc": cc, "ss": ss,
            "ksink": ksink, "vsink1": vsink1,
            "bmask": bmask, "smask": smask, "diag01": diag01,
        })
    return in_maps


# ---------------- cached PJRT runner (modeled on bass2jax.run_bass_via_pjrt) ----
_CACHE = {}


def _get_runner():
    if "runner" in _CACHE:
        return _CACHE["runner"]

    import jax
    from jax.sharding import Mesh, PartitionSpec
    from jax.experimental.shard_map import shard_map

    nc = _build_program()
    bass2jax.install_neuronx_cc_hook()

    partition_name = (nc.partition_id_tensor.name
                      if nc.partition_id_tensor else None)
    in_names = []
    out_names = []
    out_avals = []
    zero_outs = []
    for alloc in nc.m.functions[0].allocations:
        if not isinstance(alloc, mybir.MemoryLocationSet):
            continue
        if not alloc.memorylocations:
            continue
        name = alloc.memorylocations[0].name
        if alloc.kind == "ExternalInput":
            if name != partition_name:
                in_names.append(name)
        elif alloc.kind == "ExternalOutput":
            shape = tuple(alloc.tensor_shape)
            dtype = mybir.dt.np(alloc.dtype)
            out_names.append(name)
            out_avals.append(jax.core.ShapedArray(shape, dtype))
            zero_outs.append(np.zeros(shape, dtype))
    n_params = len(in_names)
    n_outs = len(out_avals)
    all_names = in_names + out_names
    if partition_name is not None:
        all_names = all_names + [partition_name]
    donate = tuple(range(n_params, n_params + n_outs))

    def _body(*args):
        operands = list(args)
        if partition_name is not None:
            operands.append(bass2jax.partition_id_tensor())
        outs = bass2jax._bass_exec_p.bind(
            *operands,
            out_avals=tuple(out_avals),
            in_names=tuple(all_names),
            out_names=tuple(out_names),
            lowering_input_output_aliases=(),
            sim_require_finite=True,
            sim_require_nnan=True,
            nc=nc,
        )
        return tuple(outs)

    devices = jax.devices()[:NCORES]
    mesh = Mesh(np.asarray(devices), ("core",))
    in_specs = (PartitionSpec("core"),) * (n_params + n_outs)
    out_specs = (PartitionSpec("core"),) * n_outs
    sharded = jax.jit(
        shard_map(_body, mesh=mesh, in_specs=in_specs, out_specs=out_specs,
                  check_rep=False),
        donate_argnums=donate, keep_unused=True,
    )

    def run(in_maps):
        concat_in = [
            np.concatenate([np.asarray(in_maps[c][nm]) for c in range(NCORES)],
                           axis=0)
            for nm in in_names
        ]
        concat_zeros = [
            np.zeros((NCORES * z.shape[0], *z.shape[1:]), z.dtype)
            for z in zero_outs
        ]
        out_arrs = sharded(*concat_in, *concat_zeros)
        return [
            {nm: np.asarray(out_arrs[i]).reshape(NCORES, *out_avals[i].shape)[c]
             for i, nm in enumerate(out_names)}
            for c in range(NCORES)
        ]

    _CACHE["runner"] = run
    _CACHE["internals"] = {
        "nc": nc, "sharded": sharded, "mesh": mesh,
        "in_names": in_names, "out_names": out_names,
        "out_avals": out_avals, "zero_outs": zero_outs,
        "body": _body, "n_params": n_params,
    }
    return run


def kernel(x, cos, sin, W_attn, W_proj, k_sink, v_sink):
    run = _get_runner()
    in_maps = _prep_inputs(x, cos, sin, W_attn, W_proj, k_sink, v_sink)
    results = run(in_maps)
    y = np.empty((T, C), dtype=np.float32)
    for c in range(NCORES):
        y[:, 256 * c: 256 * (c + 1)] = results[c]["yT"].T
    return y.reshape(1, T, C)
